# revision 23
# baseline (speedup 1.0000x reference)
"""Trainium2 Bass kernel for a pre-norm transformer block with patch-local
(serialized-order) attention.

Strategy: the whole block is row-independent except attention, which mixes
rows only within contiguous 128-row patches of the *serialized* order.  So we
gather feat by `order` on the host, shard the serialized rows across the 8
cores (128 patches/core), run the entire block per-core with zero cross-core
traffic, and scatter back on the host.

Per-core dataflow (row-major f32 residual stream, bf16 matmul operands):
  LN1 (bn_stats, batched stats) -> xn bf16 (ACT) -> PE-transpose -> xnT
  qkT = Wqk'(stationary) @ xnT        [j, rows] (LN gain + attn scale folded)
  v   = xnT(stationary) @ Wv'         [rows, 8, 33] row-major + ones col
  S^T group = 4 row-tiled concurrent MMs into one [128,512] psum; one Exp
  [O'_h | denom_h] = A^T_h(stat) @ [v_h | ones]   row-major + softmax denom
  attnO = O' * recip(denom)  (one broadcast tensor_tensor per group)
  proj: y = attnO^T(stat) @ Wp, x1 = feat + y
  LN2 -> xn2T; h1T = W1'(stat) @ xn2T; gelu; h2T = W2(stat) @ h1T;
  transpose h2T; x2 = x1 + h2
"""

import numpy as np

N, C, H, K, HID = 131072, 256, 8, 128, 1024
D = C // H          # 32
NCORES = 8
R = N // NCORES     # 16384 rows per core
SCALE = D ** -0.5

CPC = 4             # patches per chunk
RW = K * CPC        # 512 rows per chunk

_CACHE = {}


def _build(n_chunks, has_qkbias, has_vbias, has_pbias):
    from contextlib import ExitStack
    import concourse.bass as bass
    import concourse.bacc as bacc
    import concourse.tile as tile
    from concourse import mybir
    from concourse.bass import broadcast_tensor_aps
    from concourse.masks import make_identity

    f32 = mybir.dt.float32
    bf16 = mybir.dt.bfloat16
    AF = mybir.ActivationFunctionType
    OP = mybir.AluOpType

    rows_total = RW * n_chunks

    nc = bacc.Bacc()
    feat = nc.dram_tensor("feat", [rows_total, C], f32, kind="ExternalInput")
    wqk = nc.dram_tensor("wqk", [2, 128, 512], bf16, kind="ExternalInput")
    wv = nc.dram_tensor("wv", [2, 128, 256], bf16, kind="ExternalInput")
    wp = nc.dram_tensor("wp", [2, 128, 256], bf16, kind="ExternalInput")
    w1 = nc.dram_tensor("w1", [2, 128, 1024], bf16, kind="ExternalInput")
    w2 = nc.dram_tensor("w2", [8, 128, 256], bf16, kind="ExternalInput")
    bqk = nc.dram_tensor("bqk", [128, 4], f32, kind="ExternalInput")
    b1f = nc.dram_tensor("b1f", [128, 8], f32, kind="ExternalInput")
    if has_vbias:
        bv = nc.dram_tensor("bv", [1, 256], f32, kind="ExternalInput")
    if has_pbias:
        bp = nc.dram_tensor("bp", [1, 256], f32, kind="ExternalInput")
    out = nc.dram_tensor("out", [rows_total, C], f32, kind="ExternalOutput")

    with ExitStack() as ctx:
        tc = ctx.enter_context(tile.TileContext(nc))

        const = ctx.enter_context(tc.tile_pool(name="const", bufs=1))
        wqk_sb = const.tile([128, 2, 512], bf16)
        wv_sb = const.tile([128, 2, 256], bf16)
        wp_sb = const.tile([128, 2, 256], bf16)
        w1_sb = const.tile([128, 2, 1024], bf16)
        w2_sb = const.tile([128, 8, 256], bf16)
        for cb in range(2):
            nc.sync.dma_start(wqk_sb[:, cb, :], wqk[cb])
            nc.sync.dma_start(wv_sb[:, cb, :], wv[cb])
            nc.sync.dma_start(wp_sb[:, cb, :], wp[cb])
            nc.sync.dma_start(w1_sb[:, cb, :], w1[cb])
        for hb in range(8):
            nc.sync.dma_start(w2_sb[:, hb, :], w2[hb])
        bqk_sb = const.tile([128, 4], f32)
        nc.sync.dma_start(bqk_sb, bqk[:, :])
        b1f_sb = const.tile([128, 8], f32)
        nc.sync.dma_start(b1f_sb, b1f[:, :])
        if has_vbias:
            bv_sb = const.tile([128, 256], f32)
            nc.sync.dma_start(
                bv_sb, bass.AP(tensor=bv.tensor, offset=bv.offset,
                               ap=[[0, 128]] + list(bv.ap[1:])))
        if has_pbias:
            bp_sb = const.tile([128, 256], f32)
            nc.sync.dma_start(
                bp_sb, bass.AP(tensor=bp.tensor, offset=bp.offset,
                               ap=[[0, 128]] + list(bp.ap[1:])))
        eps_t = const.tile([128, 1], f32)
        nc.vector.memset(eps_t, 1e-5)

        feat_p = ctx.enter_context(tc.tile_pool(name="feat", bufs=10))
        small = ctx.enter_context(tc.tile_pool(name="small", bufs=10))
        xn_p = ctx.enter_context(tc.tile_pool(name="xn", bufs=4))
        xnT_p = ctx.enter_context(tc.tile_pool(name="xnT", bufs=2))
        qkT_p = ctx.enter_context(tc.tile_pool(name="qkT", bufs=2))
        v_p = ctx.enter_context(tc.tile_pool(name="v", bufs=4))
        at_p = ctx.enter_context(tc.tile_pool(name="at", bufs=3))
        ao_p = ctx.enter_context(tc.tile_pool(name="ao", bufs=3))
        x1_p = ctx.enter_context(tc.tile_pool(name="x1", bufs=8))
        h1_p = ctx.enter_context(tc.tile_pool(name="h1", bufs=2))
        h2_p = ctx.enter_context(tc.tile_pool(name="h2", bufs=2))
        out_p = ctx.enter_context(tc.tile_pool(name="xout", bufs=3))

        psBig = ctx.enter_context(tc.tile_pool(name="psBig", bufs=2, space="PSUM"))
        psMed = ctx.enter_context(tc.tile_pool(name="psMed", bufs=2, space="PSUM"))
        psSt = ctx.enter_context(tc.tile_pool(name="psSt", bufs=1, space="PSUM"))

        def layernorm_to_T(src_tiles, dst_T):
            """src: 4 row-major f32 [128,256] tiles -> normalized bf16
            transposed dst_T [128, CPC, 2, 128] (patch, c-blk, rows);
            stats batched across patches, transposes via DMA xbar."""
            mvc = small.tile([128, CPC, 2], f32, tag="mv")
            for p, ft in enumerate(src_tiles):
                stats = small.tile([128, 6], f32, tag="stats")
                nc.vector.bn_stats(stats, ft)
                nc.vector.bn_aggr(mvc[:, p, :], stats)
            rs = small.tile([128, CPC], f32, tag="rs")
            nc.scalar.activation(rs, mvc[:, :, 1], AF.Sqrt, bias=eps_t, scale=1.0)
            nc.vector.reciprocal(rs, rs)
            nmrs = small.tile([128, CPC], f32, tag="nmrs")
            nc.vector.scalar_tensor_tensor(
                out=nmrs, in0=mvc[:, :, 0], scalar=-1.0, in1=rs,
                op0=OP.mult, op1=OP.mult)
            for p, ft in enumerate(src_tiles):
                xn = xn_p.tile([128, 256], bf16, tag="xn")
                nc.scalar.activation(xn, ft, AF.Identity,
                                     bias=nmrs[:, p:p + 1], scale=rs[:, p:p + 1])
                nc.sync.dma_start_transpose(dst_T[:, p], xn)

        for ci in range(n_chunks):
            r0 = ci * RW

            # ---- load + LN1 ----
            fts = []
            for p in range(CPC):
                ft = feat_p.tile([128, 256], f32, tag="feat")
                nc.sync.dma_start(ft, feat[r0 + 128 * p: r0 + 128 * (p + 1), :])
                fts.append(ft)
            xnT = xnT_p.tile([128, CPC, 2, 128], bf16, tag="xnT")
            layernorm_to_T(fts, xnT)

            # ---- qkT: [j(4 blk x 128), rows] ----
            qkT = qkT_p.tile([128, 4, RW], bf16, tag="qkT")
            for jb in range(4):
                ps = psBig.tile([128, RW], f32, tag="big")
                for cb in range(2):
                    nc.tensor.matmul(
                        ps, lhsT=wqk_sb[:, cb, 128 * jb:128 * (jb + 1)],
                        rhs=xnT[:, :, cb, :], start=(cb == 0), stop=(cb == 1))
                if has_qkbias:
                    nc.vector.tensor_scalar(
                        out=qkT[:, jb, :], in0=ps, scalar1=bqk_sb[:, jb:jb + 1],
                        scalar2=None, op0=OP.add)
                else:
                    nc.vector.tensor_copy(qkT[:, jb, :], ps)

            # ---- v row-major per patch: [128 rows, 8, 33] (ones col) ----
            vs = []
            for p in range(CPC):
                vt = v_p.tile([128, 8, 33], bf16, tag="v")
                nc.vector.memset(vt[:, :, 32:33], 1.0)
                ps = psMed.tile([128, 256], f32, tag="med")
                if has_vbias:
                    nc.vector.tensor_copy(ps, bv_sb)
                for cb in range(2):
                    nc.tensor.matmul(
                        ps, lhsT=xnT[:, p, cb, :],
                        rhs=wv_sb[:, cb, :],
                        start=(cb == 0 and not has_vbias), stop=(cb == 1))
                nc.vector.tensor_copy(
                    vt[:, :, 0:32], ps.rearrange("p (h d) -> p h d", h=8))
                vs.append(vt)

            # ---- attention + proj + residual per patch ----
            x1s = []
            for p in range(CPC):
                rsl = slice(128 * p, 128 * (p + 1))
                attnO = ao_p.tile([128, 256], bf16, tag="ao")
                for g in range(2):          # 4-head groups
                    qb = g
                    # one PSUM bank per head: concurrent row-tiled matmuls
                    # must not write the same bank
                    st = psSt.tile([128, 4, 512], f32, tag="st")
                    for hh in range(4):
                        po = slice(32 * hh, 32 * (hh + 1))
                        nc.tensor.matmul(
                            st[:, hh, 0:128],
                            lhsT=qkT[po, 2 + qb, rsl],
                            rhs=qkT[po, qb, rsl], start=True, stop=True,
                            tile_position=(32 * hh, 0))
                    atg = at_p.tile([128, 512], bf16, tag="at")
                    nc.scalar.activation(
                        atg.rearrange("p (h e) -> p h e", h=4),
                        st[:, :, 0:128], AF.Exp)
                    pav = psMed.tile([128, 132], f32, tag="med")
                    for hh in range(4):
                        h = 4 * g + hh
                        nc.tensor.matmul(
                            pav[:, 33 * hh:33 * hh + 33],
                            lhsT=atg[:, 128 * hh:128 * (hh + 1)],
                            rhs=vs[p][:, h, :], start=True, stop=True)
                    pav3 = pav.rearrange("p (h e) -> p h e", h=4)
                    rc = small.tile([128, 4, 1], f32, tag="rc")
                    nc.vector.reciprocal(rc, pav3[:, :, 32:33])
                    ob, rcb = broadcast_tensor_aps(pav3[:, :, 0:32], rc[:, :, :])
                    nc.vector.tensor_tensor(
                        out=attnO[:, 128 * g:128 * (g + 1)].rearrange(
                            "p (h d) -> p h d", h=4),
                        in0=ob, in1=rcb, op=OP.mult)
                # transpose attnO -> OT via DMA xbar
                oT = xn_p.tile([128, 2, 128], bf16, tag="oT")
                nc.sync.dma_start_transpose(oT[:, :, :], attnO)
                # proj + residual
                psy = psMed.tile([128, 256], f32, tag="med")
                if has_pbias:
                    nc.vector.tensor_copy(psy, bp_sb)
                for cb in range(2):
                    nc.tensor.matmul(
                        psy, lhsT=oT[:, cb, :], rhs=wp_sb[:, cb, :],
                        start=(cb == 0 and not has_pbias), stop=(cb == 1))
                x1 = x1_p.tile([128, 256], f32, tag="x1")
                nc.vector.tensor_add(x1, fts[p], psy)
                x1s.append(x1)

            # ---- LN2 -> xn2T ----
            xn2T = xnT_p.tile([128, CPC, 2, 128], bf16, tag="xn2T")
            layernorm_to_T(x1s, xn2T)

            # ---- MLP: h1T (gelu) then h2T, transpose back ----
            h1g = h1_p.tile([128, 8, RW], bf16, tag="h1")
            for jb in range(8):
                ps = psBig.tile([128, RW], f32, tag="big")
                for cb in range(2):
                    nc.tensor.matmul(
                        ps, lhsT=w1_sb[:, cb, 128 * jb:128 * (jb + 1)],
                        rhs=xn2T[:, :, cb, :], start=(cb == 0), stop=(cb == 1))
                nc.scalar.activation(
                    h1g[:, jb, :], ps, AF.Gelu_apprx_tanh,
                    bias=b1f_sb[:, jb:jb + 1], scale=1.0)
            h2T = h2_p.tile([128, 2, RW], bf16, tag="h2T")
            for cb in range(2):
                ps = psBig.tile([128, RW], f32, tag="big")
                for hb in range(8):
                    nc.tensor.matmul(
                        ps, lhsT=w2_sb[:, hb, 128 * cb:128 * (cb + 1)],
                        rhs=h1g[:, hb, :], start=(hb == 0), stop=(hb == 7))
                nc.vector.tensor_copy(h2T[:, cb, :], ps)
            for p in range(CPC):
                h2r = out_p.tile([128, 256], bf16, tag="h2r")
                for cb in range(2):
                    nc.sync.dma_start_transpose(
                        h2r[:, 128 * cb:128 * (cb + 1)],
                        h2T[:, cb, 128 * p:128 * (p + 1)])
                x2 = out_p.tile([128, 256], f32, tag="x2")
                nc.vector.tensor_add(x2, x1s[p], h2r)
                nc.sync.dma_start(
                    out[r0 + 128 * p: r0 + 128 * (p + 1), :], x2)

    nc.finalize()
    return nc


def _prep_weights(ln1_g, ln1_b, w_qkv, b_qkv, w_proj, b_proj,
                  ln2_g, ln2_b, w1, b1, w2, b2):
    import ml_dtypes
    bf = ml_dtypes.bfloat16
    wq = (ln1_g[:, None] * w_qkv).astype(np.float32).copy()
    bq = (ln1_b @ w_qkv + b_qkv).astype(np.float32).copy()
    wq[:, :256] *= SCALE
    bq[:256] *= SCALE
    w1f = (ln2_g[:, None] * w1).astype(np.float32)
    b1f = (ln2_b @ w1 + b1).astype(np.float32)
    m = {
        "wqk": np.ascontiguousarray(wq[:, :512].reshape(2, 128, 512)).astype(bf),
        "wv": np.ascontiguousarray(wq[:, 512:768].reshape(2, 128, 256)).astype(bf),
        "wp": np.ascontiguousarray(w_proj.reshape(2, 128, 256)).astype(bf),
        "w1": np.ascontiguousarray(w1f.reshape(2, 128, 1024)).astype(bf),
        "w2": np.ascontiguousarray(w2.reshape(8, 128, 256)).astype(bf),
        "bqk": np.ascontiguousarray(bq[:512].reshape(4, 128).T).astype(np.float32),
        "b1f": np.ascontiguousarray(b1f.reshape(8, 128).T).astype(np.float32),
    }
    has_qkbias = bool(np.any(bq[:512]))
    has_vbias = bool(np.any(bq[512:768]))
    has_pbias = bool(np.any(b_proj))
    if has_vbias:
        m["bv"] = bq[512:768].reshape(1, 256).astype(np.float32)
    if has_pbias:
        m["bp"] = np.asarray(b_proj, np.float32).reshape(1, 256)
    return m, has_qkbias, has_vbias, has_pbias


PROFILE = False
LAST_EXEC_NS = None


def kernel(feat, ln1_g, ln1_b, w_qkv, b_qkv, w_proj, b_proj,
           ln2_g, ln2_b, w1, b1, w2, b2, order, inverse):
    global LAST_EXEC_NS
    import sys
    if "/opt/trn_rl_repo" not in sys.path:
        sys.path.insert(0, "/opt/trn_rl_repo")
    from concourse.bass_utils import run_bass_kernel_spmd

    feat = np.asarray(feat, np.float32)
    order_np = np.asarray(order)
    args = [np.asarray(a, np.float32) for a in
            (ln1_g, ln1_b, w_qkv, b_qkv, w_proj, b_proj,
             ln2_g, ln2_b, w1, b1, w2, b2)]
    wmap, has_qkbias, has_vbias, has_pbias = _prep_weights(*args)
    b2_np = args[11]

    n_chunks = R // RW
    key = (n_chunks, has_qkbias, has_vbias, has_pbias)
    if key not in _CACHE:
        _CACHE[key] = _build(*key)
    nc = _CACHE[key]

    feat_g = feat[order_np]          # serialized order
    in_maps = []
    for m in range(NCORES):
        im = dict(wmap)
        im["feat"] = feat_g[m * R:(m + 1) * R]
        in_maps.append(im)

    res = run_bass_kernel_spmd(nc, in_maps, core_ids=list(range(NCORES)),
                               trace=PROFILE)
    if PROFILE:
        LAST_EXEC_NS = res.exec_time_ns
    out_serial = np.concatenate([res.results[m]["out"] for m in range(NCORES)],
                                axis=0)
    out_serial = out_serial + b2_np[None, :]
    final = np.empty((N, C), np.float32)
    final[order_np] = out_serial
    return final


# revision 29
# speedup vs baseline: 1.4183x; 1.4183x over previous
"""Trainium2 Bass kernel for a pre-norm transformer block with patch-local
(serialized-order) attention.

Strategy: the whole block is row-independent except attention, which mixes
rows only within contiguous 128-row patches of the *serialized* order.  So we
gather feat by `order` on the host, shard the serialized rows across the 8
cores (128 patches/core), run the entire block per-core with zero cross-core
traffic, and scatter back on the host.

Per-core dataflow (row-major f32 residual stream, bf16 matmul operands):
  LN1 (bn_stats, batched stats) -> xn bf16 (ACT) -> PE-transpose -> xnT
  qkT = Wqk'(stationary) @ xnT        [j, rows] (LN gain + attn scale folded)
  v   = xnT(stationary) @ Wv'         [rows, 8, 33] row-major + ones col
  S^T group = 4 row-tiled concurrent MMs into one [128,512] psum; one Exp
  [O'_h | denom_h] = A^T_h(stat) @ [v_h | ones]   row-major + softmax denom
  attnO = O' * recip(denom)  (one broadcast tensor_tensor per group)
  proj: y = attnO^T(stat) @ Wp, x1 = feat + y
  LN2 -> xn2T; h1T = W1'(stat) @ xn2T; gelu; h2T = W2(stat) @ h1T;
  transpose h2T; x2 = x1 + h2
"""

import numpy as np

N, C, H, K, HID = 131072, 256, 8, 128, 1024
D = C // H          # 32
NCORES = 8
R = N // NCORES     # 16384 rows per core
SCALE = D ** -0.5

CPC = 4             # patches per chunk
RW = K * CPC        # 512 rows per chunk

_CACHE = {}


def _build(n_chunks, has_qkbias, has_vbias, has_pbias):
    from contextlib import ExitStack
    import concourse.bass as bass
    import concourse.bacc as bacc
    import concourse.tile as tile
    from concourse import mybir
    from concourse.bass import broadcast_tensor_aps
    from concourse.masks import make_identity

    f32 = mybir.dt.float32
    bf16 = mybir.dt.bfloat16
    AF = mybir.ActivationFunctionType
    OP = mybir.AluOpType

    rows_total = RW * n_chunks

    nc = bacc.Bacc()
    feat = nc.dram_tensor("feat", [rows_total, C], f32, kind="ExternalInput")
    wqk = nc.dram_tensor("wqk", [2, 128, 512], bf16, kind="ExternalInput")
    wv = nc.dram_tensor("wv", [2, 128, 256], bf16, kind="ExternalInput")
    wp = nc.dram_tensor("wp", [2, 128, 256], bf16, kind="ExternalInput")
    w1 = nc.dram_tensor("w1", [2, 128, 1024], bf16, kind="ExternalInput")
    w2 = nc.dram_tensor("w2", [8, 128, 256], bf16, kind="ExternalInput")
    bqk = nc.dram_tensor("bqk", [128, 4], f32, kind="ExternalInput")
    b1f = nc.dram_tensor("b1f", [128, 8], f32, kind="ExternalInput")
    if has_vbias:
        bv = nc.dram_tensor("bv", [1, 256], f32, kind="ExternalInput")
    if has_pbias:
        bp = nc.dram_tensor("bp", [1, 256], f32, kind="ExternalInput")
    out = nc.dram_tensor("out", [rows_total, C], f32, kind="ExternalOutput")

    with ExitStack() as ctx:
        tc = ctx.enter_context(tile.TileContext(nc))

        const = ctx.enter_context(tc.tile_pool(name="const", bufs=1))
        wqk_sb = const.tile([128, 2, 512], bf16)
        wv_sb = const.tile([128, 2, 256], bf16)
        wp_sb = const.tile([128, 2, 256], bf16)
        w1_sb = const.tile([128, 2, 1024], bf16)
        w2_sb = const.tile([128, 8, 256], bf16)
        for cb in range(2):
            nc.sync.dma_start(wqk_sb[:, cb, :], wqk[cb])
            nc.sync.dma_start(wv_sb[:, cb, :], wv[cb])
            nc.sync.dma_start(wp_sb[:, cb, :], wp[cb])
            nc.sync.dma_start(w1_sb[:, cb, :], w1[cb])
        for hb in range(8):
            nc.sync.dma_start(w2_sb[:, hb, :], w2[hb])
        bqk_sb = const.tile([128, 4], f32)
        nc.sync.dma_start(bqk_sb, bqk[:, :])
        b1f_sb = const.tile([128, 8], f32)
        nc.sync.dma_start(b1f_sb, b1f[:, :])
        if has_vbias:
            bv_sb = const.tile([128, 256], f32)
            nc.sync.dma_start(
                bv_sb, bass.AP(tensor=bv.tensor, offset=bv.offset,
                               ap=[[0, 128]] + list(bv.ap[1:])))
        if has_pbias:
            bp_sb = const.tile([128, 256], f32)
            nc.sync.dma_start(
                bp_sb, bass.AP(tensor=bp.tensor, offset=bp.offset,
                               ap=[[0, 128]] + list(bp.ap[1:])))
        ident = const.tile([128, 128], bf16)
        make_identity(nc, ident)
        eps_t = const.tile([128, 1], f32)
        nc.vector.memset(eps_t, 1e-5)

        feat_p = ctx.enter_context(tc.tile_pool(name="feat", bufs=10))
        small = ctx.enter_context(tc.tile_pool(name="small", bufs=10))
        xn_p = ctx.enter_context(tc.tile_pool(name="xn", bufs=4))
        xnT_p = ctx.enter_context(tc.tile_pool(name="xnT", bufs=2))
        qkT_p = ctx.enter_context(tc.tile_pool(name="qkT", bufs=2))
        v_p = ctx.enter_context(tc.tile_pool(name="v", bufs=4))
        at_p = ctx.enter_context(tc.tile_pool(name="at", bufs=3))
        ao_p = ctx.enter_context(tc.tile_pool(name="ao", bufs=3))
        x1_p = ctx.enter_context(tc.tile_pool(name="x1", bufs=8))
        h1_p = ctx.enter_context(tc.tile_pool(name="h1", bufs=2))
        h2_p = ctx.enter_context(tc.tile_pool(name="h2", bufs=2))
        out_p = ctx.enter_context(tc.tile_pool(name="xout", bufs=3))

        psQK = ctx.enter_context(tc.tile_pool(name="psQK", bufs=1, space="PSUM"))
        psMLP = ctx.enter_context(tc.tile_pool(name="psMLP", bufs=3, space="PSUM"))
        psMed = ctx.enter_context(tc.tile_pool(name="psMed", bufs=1, space="PSUM"))
        psT = ctx.enter_context(tc.tile_pool(name="psT", bufs=1, space="PSUM"))
        psSt = ctx.enter_context(tc.tile_pool(name="psSt", bufs=1, space="PSUM"))

        def ln_stats(ft, mvc, p):
            stats = small.tile([128, 6], f32, tag="stats")
            nc.vector.bn_stats(stats, ft)
            nc.vector.bn_aggr(mvc[:, p, :], stats)

        def ln_finish(src_tiles, mvc, dst_T):
            rs = small.tile([128, CPC], f32, tag="rs")
            nc.scalar.activation(rs, mvc[:, :, 1], AF.Sqrt, bias=eps_t, scale=1.0)
            nc.vector.reciprocal(rs, rs)
            nmrs = small.tile([128, CPC], f32, tag="nmrs")
            nc.vector.scalar_tensor_tensor(
                out=nmrs, in0=mvc[:, :, 0], scalar=-1.0, in1=rs,
                op0=OP.mult, op1=OP.mult)
            for p, ft in enumerate(src_tiles):
                xn = xn_p.tile([128, 256], bf16, tag="xn")
                nc.scalar.activation(xn, ft, AF.Identity,
                                     bias=nmrs[:, p:p + 1], scale=rs[:, p:p + 1])
                for cb in range(2):
                    tp = psT.tile([128, 128], bf16, tag="tp")
                    nc.tensor.transpose(tp, xn[:, 128 * cb:128 * (cb + 1)], ident)
                    nc.vector.tensor_copy(
                        dst_T[:, cb, 128 * p:128 * (p + 1)], tp)

        def load_ln1(ci):
            r0 = ci * RW
            fts = []
            for p in range(CPC):
                ft = feat_p.tile([128, 256], f32, tag="feat")
                nc.sync.dma_start(ft, feat[r0 + 128 * p: r0 + 128 * (p + 1), :])
                fts.append(ft)
            xnT = xnT_p.tile([128, 2, RW], bf16, tag="xnT")
            mvc = small.tile([128, CPC, 2], f32, tag="mv1")
            for p, ft in enumerate(fts):
                ln_stats(ft, mvc, p)
            ln_finish(fts, mvc, xnT)
            return fts, xnT

        nxt = load_ln1(0)
        for ci in range(n_chunks):
            r0 = ci * RW
            fts, xnT = nxt

            # ---- qkT: [j(4 blk x 128), rows] ----
            qkT = qkT_p.tile([128, 4, RW], bf16, tag="qkT")
            for jb in range(4):
                ps = psQK.tile([128, RW], f32, tag="qk")
                for cb in range(2):
                    nc.tensor.matmul(
                        ps, lhsT=wqk_sb[:, cb, 128 * jb:128 * (jb + 1)],
                        rhs=xnT[:, cb, :], start=(cb == 0), stop=(cb == 1))
                if has_qkbias:
                    nc.vector.tensor_scalar(
                        out=qkT[:, jb, :], in0=ps, scalar1=bqk_sb[:, jb:jb + 1],
                        scalar2=None, op0=OP.add)
                else:
                    nc.vector.tensor_copy(qkT[:, jb, :], ps)

            # ---- v row-major per patch: [128 rows, 8, 33] (ones col) ----
            vs = []
            for p in range(CPC):
                vt = v_p.tile([128, 8, 33], bf16, tag="v")
                nc.vector.memset(vt[:, :, 32:33], 1.0)
                ps = psMed.tile([128, 256], f32, tag="med")
                if has_vbias:
                    nc.vector.tensor_copy(ps, bv_sb)
                for cb in range(2):
                    nc.tensor.matmul(
                        ps, lhsT=xnT[:, cb, 128 * p:128 * (p + 1)],
                        rhs=wv_sb[:, cb, :],
                        start=(cb == 0 and not has_vbias), stop=(cb == 1))
                nc.vector.tensor_copy(
                    vt[:, :, 0:32], ps.rearrange("p (h d) -> p h d", h=8))
                vs.append(vt)

            # ---- attention + proj + residual per patch ----
            mvc2 = small.tile([128, CPC, 2], f32, tag="mv2")
            x1s = []
            for p in range(CPC):
                rsl = slice(128 * p, 128 * (p + 1))
                attnO = ao_p.tile([128, 256], bf16, tag="ao")
                for g in range(2):          # 4-head groups
                    qb = g
                    # one PSUM bank per head: concurrent row-tiled matmuls
                    # must not write the same bank
                    st = psSt.tile([128, 4, 512], f32, tag="st")
                    for hh in range(4):
                        po = slice(32 * hh, 32 * (hh + 1))
                        nc.tensor.matmul(
                            st[:, hh, 0:128],
                            lhsT=qkT[po, 2 + qb, rsl],
                            rhs=qkT[po, qb, rsl], start=True, stop=True,
                            tile_position=(32 * hh, 0))
                    atg = at_p.tile([128, 512], bf16, tag="at")
                    nc.scalar.activation(
                        atg.rearrange("p (h e) -> p h e", h=4),
                        st[:, :, 0:128], AF.Exp)
                    pav = psMed.tile([128, 132], f32, tag="med")
                    for hh in range(4):
                        h = 4 * g + hh
                        nc.tensor.matmul(
                            pav[:, 33 * hh:33 * hh + 33],
                            lhsT=atg[:, 128 * hh:128 * (hh + 1)],
                            rhs=vs[p][:, h, :], start=True, stop=True)
                    pav3 = pav.rearrange("p (h e) -> p h e", h=4)
                    rc = small.tile([128, 4, 1], f32, tag="rc")
                    nc.vector.reciprocal(rc, pav3[:, :, 32:33])
                    ob, rcb = broadcast_tensor_aps(pav3[:, :, 0:32], rc[:, :, :])
                    nc.vector.tensor_tensor(
                        out=attnO[:, 128 * g:128 * (g + 1)].rearrange(
                            "p (h d) -> p h d", h=4),
                        in0=ob, in1=rcb, op=OP.mult)
                # transpose attnO -> OT via DMA xbar
                oT = xn_p.tile([128, 2, 128], bf16, tag="oT")
                nc.sync.dma_start_transpose(oT[:, :, :], attnO)
                # proj + residual
                psy = psMed.tile([128, 256], f32, tag="med")
                if has_pbias:
                    nc.vector.tensor_copy(psy, bp_sb)
                for cb in range(2):
                    nc.tensor.matmul(
                        psy, lhsT=oT[:, cb, :], rhs=wp_sb[:, cb, :],
                        start=(cb == 0 and not has_pbias), stop=(cb == 1))
                x1 = x1_p.tile([128, 256], f32, tag="x1")
                nc.vector.tensor_add(x1, fts[p], psy)
                ln_stats(x1, mvc2, p)
                x1s.append(x1)

            # ---- prefetch + LN1 of next chunk (overlaps this chunk's MLP) ----
            if ci + 1 < n_chunks:
                nxt = load_ln1(ci + 1)

            # ---- LN2 -> xn2T ----
            xn2T = xnT_p.tile([128, 2, RW], bf16, tag="xn2T")
            ln_finish(x1s, mvc2, xn2T)

            # ---- MLP: h1T (gelu) then h2T, transpose back ----
            h1g = h1_p.tile([128, 8, RW], bf16, tag="h1")
            for jb in range(8):
                ps = psBig.tile([128, RW], f32, tag="big")
                for cb in range(2):
                    nc.tensor.matmul(
                        ps, lhsT=w1_sb[:, cb, 128 * jb:128 * (jb + 1)],
                        rhs=xn2T[:, :, cb, :], start=(cb == 0), stop=(cb == 1))
                nc.scalar.activation(
                    h1g[:, jb, :], ps, AF.Gelu_apprx_tanh,
                    bias=b1f_sb[:, jb:jb + 1], scale=1.0)
            h2T = h2_p.tile([128, 2, RW], bf16, tag="h2T")
            for cb in range(2):
                ps = psBig.tile([128, RW], f32, tag="big")
                for hb in range(8):
                    nc.tensor.matmul(
                        ps, lhsT=w2_sb[:, hb, 128 * cb:128 * (cb + 1)],
                        rhs=h1g[:, hb, :], start=(hb == 0), stop=(hb == 7))
                nc.vector.tensor_copy(h2T[:, cb, :], ps)
            for p in range(CPC):
                h2r = out_p.tile([128, 256], bf16, tag="h2r")
                for cb in range(2):
                    nc.sync.dma_start_transpose(
                        h2r[:, 128 * cb:128 * (cb + 1)],
                        h2T[:, cb, 128 * p:128 * (p + 1)])
                x2 = out_p.tile([128, 256], f32, tag="x2")
                nc.vector.tensor_add(x2, x1s[p], h2r)
                nc.sync.dma_start(
                    out[r0 + 128 * p: r0 + 128 * (p + 1), :], x2)

    nc.finalize()
    return nc


def _prep_weights(ln1_g, ln1_b, w_qkv, b_qkv, w_proj, b_proj,
                  ln2_g, ln2_b, w1, b1, w2, b2):
    import ml_dtypes
    bf = ml_dtypes.bfloat16
    wq = (ln1_g[:, None] * w_qkv).astype(np.float32).copy()
    bq = (ln1_b @ w_qkv + b_qkv).astype(np.float32).copy()
    wq[:, :256] *= SCALE
    bq[:256] *= SCALE
    w1f = (ln2_g[:, None] * w1).astype(np.float32)
    b1f = (ln2_b @ w1 + b1).astype(np.float32)
    m = {
        "wqk": np.ascontiguousarray(wq[:, :512].reshape(2, 128, 512)).astype(bf),
        "wv": np.ascontiguousarray(wq[:, 512:768].reshape(2, 128, 256)).astype(bf),
        "wp": np.ascontiguousarray(w_proj.reshape(2, 128, 256)).astype(bf),
        "w1": np.ascontiguousarray(w1f.reshape(2, 128, 1024)).astype(bf),
        "w2": np.ascontiguousarray(w2.reshape(8, 128, 256)).astype(bf),
        "bqk": np.ascontiguousarray(bq[:512].reshape(4, 128).T).astype(np.float32),
        "b1f": np.ascontiguousarray(b1f.reshape(8, 128).T).astype(np.float32),
    }
    has_qkbias = bool(np.any(bq[:512]))
    has_vbias = bool(np.any(bq[512:768]))
    has_pbias = bool(np.any(b_proj))
    if has_vbias:
        m["bv"] = bq[512:768].reshape(1, 256).astype(np.float32)
    if has_pbias:
        m["bp"] = np.asarray(b_proj, np.float32).reshape(1, 256)
    return m, has_qkbias, has_vbias, has_pbias


PROFILE = False
LAST_EXEC_NS = None


def kernel(feat, ln1_g, ln1_b, w_qkv, b_qkv, w_proj, b_proj,
           ln2_g, ln2_b, w1, b1, w2, b2, order, inverse):
    global LAST_EXEC_NS
    import sys
    if "/opt/trn_rl_repo" not in sys.path:
        sys.path.insert(0, "/opt/trn_rl_repo")
    from concourse.bass_utils import run_bass_kernel_spmd

    feat = np.asarray(feat, np.float32)
    order_np = np.asarray(order)
    args = [np.asarray(a, np.float32) for a in
            (ln1_g, ln1_b, w_qkv, b_qkv, w_proj, b_proj,
             ln2_g, ln2_b, w1, b1, w2, b2)]
    wmap, has_qkbias, has_vbias, has_pbias = _prep_weights(*args)
    b2_np = args[11]

    n_chunks = R // RW
    key = (n_chunks, has_qkbias, has_vbias, has_pbias)
    if key not in _CACHE:
        _CACHE[key] = _build(*key)
    nc = _CACHE[key]

    feat_g = feat[order_np]          # serialized order
    in_maps = []
    for m in range(NCORES):
        im = dict(wmap)
        im["feat"] = feat_g[m * R:(m + 1) * R]
        in_maps.append(im)

    res = run_bass_kernel_spmd(nc, in_maps, core_ids=list(range(NCORES)),
                               trace=PROFILE)
    if PROFILE:
        LAST_EXEC_NS = res.exec_time_ns
    out_serial = np.concatenate([res.results[m]["out"] for m in range(NCORES)],
                                axis=0)
    out_serial = out_serial + b2_np[None, :]
    final = np.empty((N, C), np.float32)
    final[order_np] = out_serial
    return final
